# revision 1
# baseline (speedup 1.0000x reference)
"""ChebNet forward on 8 TRN2 NeuronCores — pure data parallelism over batch.

Per-core layout: activations in "layout T" [(b_local*16+ch) = 128 partitions,
node = 307]; 8 batch per 128-partition group, 2 groups per pair tile
[128, 614]; 4 pairs per core (64 batch). Channel matmuls use block-diagonal
kron(I8, W) stationary operands in bf16 (fp32 PSUM accumulate; measured
7.8e-3 max rel err, matching host emulation). Laplacian matmuls go through
a PE transpose to layout A [node-chunk, (b,ch)]. The fp32 state copy for the
per-iteration identity-add is fused into the DVE drains, which keeps the
iteration-to-iteration error from random-walking through bf16 roundings.

Engine budget per core (cost model): ACT (tanh/sigmoid + some copies) is the
bottleneck at ~600us busy; PE ~480us; DVE ~400us; GPSIMD takes elementwise
adds. Emission is stage-interleaved across pairs (GL in waves of 2, NL in
halves) so each engine streams same-kind work back-to-back instead of
serializing through per-pair chains. Host precomputes L/Chebyshev polys and
all block-diag weights, and re-assembles the output layout after the run.
"""
import sys

sys.path.insert(0, "/opt/trn_rl_repo")

import contextlib
from contextlib import ExitStack

import numpy as np

import concourse.bass as bass
import concourse.mybir as mybir
from concourse import bacc, tile
from concourse.bass_utils import run_bass_kernel_spmd

# ---- problem constants (hardcoded; kernel.py must be self-contained) ----
B, N, C_IN, H, K, LSTM_H = 512, 307, 5, 16, 6, 32
NCORES = 8
BL = B // NCORES            # 64 batch per core
NPAIR = BL // 16            # 4 pairs of (2 groups x 8 batch)
GL_ITERS = 10
NL_ITERS = 10
MCS = [128, 128, 51]        # node-dim chunks (307 = 128+128+51)
MOS = [0, 128, 256]
F32 = mybir.dt.float32
F32R = mybir.dt.float32r
BF16 = mybir.dt.bfloat16
AF = mybir.ActivationFunctionType
ALU = mybir.AluOpType

# big const dram layout [128, 24*307]: [lap x3 | q(k,mc) x18 | i307 x3 | pad]
CB_LAP, CB_Q, CB_I, CB_COLS = 0, 3, 21, 24
# wtile [128, NW*128] block-diag / identity weights
(W_GLOUT, W_GLFK, W_T, W_TN, W_U, W_UN, W_I, W_IQ, W_CW, W_C1W,
 W_GI_LO, W_GI_HI, W_GO_LO, W_GO_HI, W_GG_LO, W_GG_HI,
 W_W2LO, W_W2HI, W_W2LO5, W_W2HI5) = range(20)
NW = 20
# bvec columns
BV_2B, BV_HB, BV_CHEB, BV_CB, BV_C1B, BV_ZERO = range(6)
NBV = 8

_COMPILED = None


def _host_consts(inputs):
    """Precompute everything batch-independent, in the exact SBUF layouts."""
    f32 = np.float32
    E = np.asarray(inputs["node_emb"], f32)
    En = E / np.linalg.norm(E, axis=1, keepdims=True)
    G = (En @ En.T).astype(f32)
    d = (G.sum(-1) ** -0.5).astype(f32)
    I = np.eye(N, dtype=f32)
    L = (I - (d[:, None] * G) * d[None, :]).astype(f32)
    Ts = [I, L]
    for _ in range(K - 2):
        Ts.append((2.0 * (L @ Ts[-1]) - Ts[-2]).astype(f32))

    def mchunks(mat):  # [N, N] -> [3, 128, N] zero-padded row chunks
        out = np.zeros((3, 128, N), f32)
        for j in range(3):
            out[j, :MCS[j]] = mat[MOS[j]:MOS[j] + MCS[j]]
        return out

    cbig = np.zeros((128, CB_COLS * N), f32)

    def put_cb(idx, mat3):
        for j in range(3):
            cbig[:, (idx + j) * N:(idx + j + 1) * N] = mat3[j]

    put_cb(CB_LAP, mchunks(L))
    for k in range(K):
        put_cb(CB_Q + k * 3, mchunks(Ts[k]))
    put_cb(CB_I, mchunks(I))

    I8 = np.eye(8, dtype=f32)
    bd = lambda w: np.kron(I8, np.asarray(w, f32))
    gl_tz_w = np.asarray(inputs["gl_tz_w"], f32)
    Wih = np.asarray(inputs["lstm_Wih"], f32)   # [16, 4*32], gate order i,f,g,o
    nl_w2 = np.asarray(inputs["nl_w2"], f32)
    wt = np.zeros((128, NW * 128), f32)

    def setw(idx, w128):
        wt[:, idx * 128:(idx + 1) * 128] = w128

    setw(W_GLOUT, bd(inputs["gl_out"]))
    setw(W_GLFK, bd(inputs["gl_fk"]))
    setw(W_T, bd(gl_tz_w))
    setw(W_TN, bd(-gl_tz_w))
    setw(W_U, bd(0.5 * gl_tz_w))
    setw(W_UN, bd(-0.5 * gl_tz_w))
    setw(W_I, np.eye(128, dtype=f32))
    setw(W_IQ, 0.25 * np.eye(128, dtype=f32))
    setw(W_CW, bd(inputs["c_w"]))
    setw(W_C1W, bd(inputs["c1_w"]))
    gi, gg, go = Wih[:, 0:32], Wih[:, 64:96], Wih[:, 96:128]
    setw(W_GI_LO, bd(gi[:, 0:16])); setw(W_GI_HI, bd(gi[:, 16:32]))
    setw(W_GO_LO, bd(go[:, 0:16])); setw(W_GO_HI, bd(go[:, 16:32]))
    setw(W_GG_LO, bd(gg[:, 0:16])); setw(W_GG_HI, bd(gg[:, 16:32]))
    setw(W_W2LO, bd(nl_w2[0:16])); setw(W_W2HI, bd(nl_w2[16:32]))
    setw(W_W2LO5, bd(0.5 * nl_w2[0:16])); setw(W_W2HI5, bd(0.5 * nl_w2[16:32]))

    # front-end block-diag weights kron(I8, w[5,16]) -> [40, 128], duplicated
    # at partition base 64 (matmul base-partition constraint: 0/32/64)
    fw = np.zeros((104, 8 * 128), f32)
    for k in range(K):
        blk = bd(np.asarray(inputs["cheb_w"][k], f32))
        fw[0:40, k * 128:(k + 1) * 128] = blk
        fw[64:104, k * 128:(k + 1) * 128] = blk
    blk = bd(np.asarray(inputs["cou_w"], f32))
    fw[0:40, K * 128:(K + 1) * 128] = blk
    fw[64:104, K * 128:(K + 1) * 128] = blk

    bvec = np.zeros((128, NBV), f32)
    gl_tz_b = np.asarray(inputs["gl_tz_b"], f32)
    bvec[:, BV_2B] = np.tile(2.0 * gl_tz_b, 8)
    bvec[:, BV_HB] = np.tile(0.5 * gl_tz_b, 8)
    bvec[:, BV_CHEB] = np.tile(np.asarray(inputs["cheb_b"], f32), 8)
    bvec[:, BV_CB] = np.tile(np.asarray(inputs["c_b"], f32), 8)
    bvec[:, BV_C1B] = np.tile(np.asarray(inputs["c1_b"], f32), 8)

    assert not np.any(np.asarray(inputs["lstm_b"], f32)), \
        "nonzero lstm bias not supported by fast path"
    assert not np.any(gl_tz_b), \
        "nonzero gl_tz_b not supported (identity-add fused into drains)"

    import ml_dtypes
    bf = ml_dtypes.bfloat16
    wtf = np.zeros((128, 256), f32)
    wtf[:, 0:128] = np.eye(128, dtype=f32)
    wtf[:, 128:256] = 0.25 * np.eye(128, dtype=f32)
    return dict(cbig=cbig.astype(bf), wtile=wt.astype(bf), ftile=fw.astype(bf),
                bvec=bvec, wtf=wtf)


FLW = NPAIR * 104  # flow layout-A columns: per pair [A(40) | gap(24) | B(40)]


def _flow_shards(flow_x):
    """Per-core flow in layout A: [3, 128, FLW] = (chunk, node, padded (b,c))."""
    shards = []
    for c in range(NCORES):
        fl = np.asarray(flow_x[c * BL:(c + 1) * BL], np.float32)  # [64, 307, 5]
        fa = fl.transpose(1, 0, 2).reshape(N, BL * C_IN)          # [307, 320]
        pad = np.zeros((N, FLW), np.float32)
        for p in range(NPAIR):
            for g in range(2):
                grp = 2 * p + g
                pad[:, p * 104 + g * 64: p * 104 + g * 64 + 40] = \
                    fa[:, grp * 40:(grp + 1) * 40]
        out = np.zeros((3, 128, FLW), np.float32)
        for j in range(3):
            out[j, :MCS[j]] = pad[MOS[j]:MOS[j] + MCS[j]]
        import ml_dtypes
        shards.append(out.astype(ml_dtypes.bfloat16))
    return shards


def _rearr2(ap, width=None):
    """[p, 2*w] (or 2-bank psum) -> [p, 2, w] AP."""
    r = ap.rearrange("p (two f) -> p two f", two=2)
    return r if width is None else r[:, :, 0:width]


def _build(repeat=1, bench_out=False, gl_iters=None, nl_iters=None,
           pp_bufs=3, tgt_bufs=1, scr_bufs=2, st_bufs=3):
    nc = bacc.Bacc("TRN2", target_bir_lowering=False, debug=False)
    d_cbig = nc.dram_tensor("cbig", [128, CB_COLS * N], BF16, kind="ExternalInput").ap()
    d_wt = nc.dram_tensor("wtile", [128, NW * 128], BF16, kind="ExternalInput").ap()
    d_fw = nc.dram_tensor("ftile", [104, 8 * 128], BF16, kind="ExternalInput").ap()
    d_bv = nc.dram_tensor("bvec", [128, NBV], F32, kind="ExternalInput").ap()
    d_wtf = nc.dram_tensor("wtf", [128, 256], F32, kind="ExternalInput").ap()
    d_flow = nc.dram_tensor("flowA", [3, 128, FLW], BF16, kind="ExternalInput").ap()
    if bench_out:
        d_out = nc.dram_tensor("outscratch", [NL_ITERS, 128, BL * N // 8],
                               F32).ap()
        d_outx = nc.dram_tensor("out", [128, 8], F32,
                                kind="ExternalOutput").ap()
    else:
        d_out = nc.dram_tensor("out", [NL_ITERS, 128, BL * N // 8], F32,
                               kind="ExternalOutput").ap()
        d_outx = None

    gl_iters = GL_ITERS if gl_iters is None else gl_iters
    nl_iters = NL_ITERS if nl_iters is None else nl_iters
    with tile.TileContext(nc) as tc, ExitStack() as ctx:
        consts = ctx.enter_context(tc.tile_pool(name="consts", bufs=1))
        states = ctx.enter_context(tc.tile_pool(name="states", bufs=st_bufs))
        spool = ctx.enter_context(tc.tile_pool(name="spool", bufs=1))
        scr = ctx.enter_context(tc.tile_pool(name="scr", bufs=scr_bufs))
        front = ctx.enter_context(tc.tile_pool(name="front", bufs=1))
        pp = ctx.enter_context(tc.tile_pool(name="pp", bufs=pp_bufs, space="PSUM"))
        tgt = ctx.enter_context(tc.tile_pool(name="tgt", bufs=tgt_bufs, space="PSUM"))

        clap = consts.tile([128, 3 * N], BF16, tag="clap")
        wt = consts.tile([128, NW * 128], BF16, tag="wt")
        bv = consts.tile([128, NBV], F32, tag="bv")
        wtf = consts.tile([128, 256], F32, tag="wtf")
        nc.sync.dma_start(clap[:], d_cbig[:, CB_LAP * N:(CB_LAP + 3) * N])
        nc.sync.dma_start(wt[:], d_wt[:])
        nc.sync.dma_start(bv[:], d_bv[:])
        nc.sync.dma_start(wtf[:], d_wtf[:])

        def lap(j):  # laplacian rhs chunk [mcs, 307] f32r
            return clap[0:MCS[j], j * N:(j + 1) * N]

        def w(idx):  # block-diag lhsT [128, 128] f32r
            return wt[:, idx * 128:(idx + 1) * 128]

        def bvc(idx):  # bias column [128, 1]
            return bv[:, idx:idx + 1]

        def drain(dst, ps, scale=1.0, bias_idx=BV_ZERO, width=N):
            """dst[128, 2*width] (sbuf) = pair-psum * scale + bias_col."""
            nc.vector.tensor_scalar(
                _rearr2(dst[:]), _rearr2(ps[:], width), scale, bvc(bias_idx),
                ALU.mult, ALU.add)

        def bdmm(ps, widx, x, start=True, stop=True):
            """pair psum += bd-weight.T @ x for both 307-halves of x."""
            for g in range(2):
                nc.tensor.matmul(
                    ps[:, g * 512:g * 512 + N],
                    w(widx), x[:, g * N:(g + 1) * N],
                    start=start, stop=stop)

        ident = w(W_I)
        acc = None
        if bench_out:
            acc = consts.tile([128, 8], F32, tag="acc")
            nc.gpsimd.memset(acc[:], 0.0)

        def iadd_f32(ps, xf, stop=False):
            """pair psum += I @ xf (f32 state copy), both halves."""
            for g in range(2):
                nc.tensor.matmul(ps[:, g * 512:g * 512 + N],
                                 wtf[:, 0:128], xf[:, g * N:(g + 1) * N],
                                 start=False, stop=stop)
        cur, prev, s00, s11, gst, pprev = {}, {}, {}, {}, {}, {}
        curf, gstf = {}, {}

        with (tc.For_i(0, repeat, 1) if repeat > 1
              else contextlib.nullcontext()):
            if bench_out:
                nc.vector.tensor_scalar(acc[:], acc[:], 1.0, 1.0,
                                        ALU.mult, ALU.add)
            # ---------- front-end: Chebyshev conv ----------
            if True:
                cfro = front.tile([128, 21 * N], BF16, tag="cfro")
                fwt = front.tile([104, 8 * 128], BF16, tag="fwt")
                flw = front.tile([128, 3 * FLW], BF16, tag="flw")
                # order: flow + weights + identity first (unblocks the
                # flow-transpose matmuls), the 1.4MB Chebyshev block last
                for j in range(3):
                    nc.sync.dma_start(flw[:, j * FLW:(j + 1) * FLW], d_flow[j])
                nc.sync.dma_start(fwt[:], d_fw[:])
                nc.sync.dma_start(cfro[:, 18 * N:21 * N],
                                  d_cbig[:, (CB_Q + 18) * N:(CB_Q + 21) * N])
                for k in range(K):
                    nc.sync.dma_start(
                        cfro[:, k * 3 * N:(k + 1) * 3 * N],
                        d_cbig[:, (CB_Q + k * 3) * N:(CB_Q + (k + 1) * 3) * N])

                def cf(idx, j):  # q/i rhs chunk [mcs, 307] f32r (idx rel. to CB_Q)
                    return cfro[0:MCS[j], (idx + j) * N:(idx + j + 1) * N]

                for p in range(NPAIR):
                    fsl = [flw[0:MCS[j], j * FLW + p * 104: j * FLW + (p + 1) * 104]
                           for j in range(3)]
                    ft_ps = pp.tile([128, 1024], F32, tag="pp")
                    for j in range(3):
                        nc.tensor.matmul(ft_ps[0:104, 0:N], fsl[j],
                                         cf(18, j), start=(j == 0), stop=(j == 2))
                    ft_s = front.tile([104, N], BF16, tag="fts")
                    nc.vector.tensor_scalar(ft_s[:], ft_ps[0:104, 0:N], 1.0, 0.0,
                                            ALU.mult, ALU.add)
                    hks = []
                    for k in range(K):
                        hk_ps = pp.tile([128, 1024], F32, tag="pp")
                        for j in range(3):
                            nc.tensor.matmul(hk_ps[0:104, 0:N], fsl[j],
                                             cf(k * 3, j), start=(j == 0),
                                             stop=(j == 2))
                        hk_s = front.tile([104, N], BF16, tag=f"hk{k}")
                        nc.vector.tensor_scalar(hk_s[:], hk_ps[0:104, 0:N], 1.0, 0.0,
                                                ALU.mult, ALU.add)
                        hks.append(hk_s)
                    oc_ps = pp.tile([128, 1024], F32, tag="pp")
                    for g in range(2):
                        for k in range(K):
                            nc.tensor.matmul(
                                oc_ps[:, g * 512:g * 512 + N],
                                fwt[g * 64:g * 64 + 40,
                                    k * 128:(k + 1) * 128],
                                hks[k][g * 64:g * 64 + 40, :],
                                start=(k == 0), stop=(k == K - 1))
                    o0_ps = tgt.tile([128, 1024], F32, tag="tgt")
                    for g in range(2):
                        nc.tensor.matmul(
                            o0_ps[:, g * 512:g * 512 + N],
                            fwt[g * 64:g * 64 + 40,
                                K * 128:(K + 1) * 128],
                            ft_s[g * 64:g * 64 + 40, :],
                            start=True, stop=True)
                    cur[p] = states.tile([128, 2 * N], BF16, tag=f"st{p}", name=f"cur{p}")
                    drain(cur[p], oc_ps, 1.0, BV_CHEB)
                    curf[p] = states.tile([128, 2 * N], F32, tag=f"stf{p}", bufs=2, name=f"curf{p}")
                    drain(curf[p], oc_ps, 1.0, BV_CHEB)
                    prev[p] = states.tile([128, 2 * N], BF16, tag=f"st{p}", name=f"prev{p}")
                    drain(prev[p], o0_ps, 1.0, BV_ZERO)

            # ---------- GL chain (stage-interleaved waves) ----------
            WV = 2

            def gl_phase(xs, wv_w, wvn_w, tgt_ps, final):
                """Stage-emitted GL over a wave of pairs.

                xs: {p: input pair tile}; accumulates alpha*(L@x@W - o3@W)
                into tgt_ps[p] (L matmuls open each bank, wvn closes if
                final)."""
                wave = list(xs)
                vps, vs, va, s1p, t1s, s2p, t2s = {}, {}, {}, {}, {}, {}, {}
                for p in wave:
                    vps[p] = pp.tile([128, 1024], F32, tag="pp", name=f"v{p}")
                    bdmm(vps[p], wv_w, xs[p])
                for p in wave:
                    vs[p] = scr.tile([128, 2 * N], BF16, tag="v_s", name=f"vs{p}")
                    drain(vs[p], vps[p])
                trp = {}
                for p in wave:
                    trp[p] = pp.tile([128, 1024], BF16, tag="pp", name=f"tr{p}")
                    for g in range(2):
                        for j in range(3):
                            nc.tensor.transpose(
                                trp[p][0:MCS[j],
                                       g * 512 + j * 128:g * 512 + (j + 1) * 128],
                                vs[p][:, g * N + MOS[j]:g * N + MOS[j] + MCS[j]],
                                ident)
                for p in wave:
                    va[p] = scr.tile([128, 768], BF16, tag="v_a", name=f"va{p}")
                    nc.scalar.activation(_rearr2(va[p][:]),
                                         _rearr2(trp[p][:], 384), AF.Copy)
                for p in wave:
                    for g in range(2):
                        for j in range(3):
                            nc.tensor.matmul(
                                tgt_ps[p][:, g * 512:g * 512 + N],
                                va[p][0:MCS[j],
                                      g * 384 + j * 128:g * 384 + (j + 1) * 128],
                                lap(j), start=(j == 0), stop=False)
                for p in wave:
                    s1p[p] = pp.tile([128, 1024], F32, tag="pp", name=f"s1{p}")
                    bdmm(s1p[p], W_GLOUT, xs[p])
                for p in wave:
                    t1s[p] = scr.tile([128, 2 * N], BF16, tag="t1", name=f"t1{p}")
                    nc.scalar.activation(_rearr2(t1s[p][:]),
                                         _rearr2(s1p[p][:], N), AF.Tanh)
                for p in wave:
                    s2p[p] = pp.tile([128, 1024], F32, tag="pp", name=f"s2{p}")
                    bdmm(s2p[p], W_GLFK, t1s[p])
                for p in wave:
                    t2s[p] = scr.tile([128, 2 * N], BF16, tag="t2", name=f"t2{p}")
                    nc.scalar.activation(_rearr2(t2s[p][:]),
                                         _rearr2(s2p[p][:], N), AF.Tanh)
                for p in wave:
                    bdmm(tgt_ps[p], wvn_w, t2s[p], start=False, stop=final)

            for it in range(gl_iters):
                waves = [list(range(w0, min(w0 + WV, NPAIR)))
                         for w0 in range(0, NPAIR, WV)]

                def gl_mid(wave, tps):
                    t2l, argl = {}, {}
                    for p in wave:
                        t2l[p] = scr.tile([128, 2 * N], BF16, tag="t2x",
                                          bufs=5, name=f"t2x{p}")
                        drain(t2l[p], tps[p], 2.0, BV_2B)     # 2*t (incl. 2b)
                    for p in wave:
                        argl[p] = scr.tile([128, 2 * N], BF16, tag="arg",
                                           bufs=5, name=f"arg{p}")
                        nc.gpsimd.tensor_add(argl[p][:], prev[p][:],
                                             t2l[p][:])
                    return t2l, argl

                def gl_fin(wave, nps, t2l):
                    for p in wave:
                        bdmm(nps[p], W_IQ, t2l[p], start=False, stop=True)
                    for p in wave:
                        # nxtf = psum_n + curf (identity-add fused into drain;
                        # gl_tz_b asserted zero in kernel())
                        nxtf = states.tile([128, 2 * N], F32, tag=f"stf{p}",
                                           bufs=2, name=f"nxtf{p}")
                        nc.vector.tensor_add(_rearr2(nxtf[:]),
                                             _rearr2(nps[p][:], N),
                                             _rearr2(curf[p][:]))
                        nxt = states.tile([128, 2 * N], BF16, tag=f"st{p}",
                                          name=f"nxt{p}")
                        nc.vector.tensor_scalar(nxt[:], nxtf[:], 1.0, 0.0,
                                                ALU.mult, ALU.add)
                        prev[p], cur[p], curf[p] = cur[p], nxt, nxtf

                wA, wB = waves
                tpsA = {p: tgt.tile([128, 1024], F32, tag="tgt",
                                    name=f"tps{p}") for p in wA}
                gl_phase({p: cur[p] for p in wA}, W_T, W_TN, tpsA, final=True)
                t2A, argA = gl_mid(wA, tpsA)
                tpsB = {p: tgt.tile([128, 1024], F32, tag="tgt",
                                    name=f"tps{p}") for p in wB}
                gl_phase({p: cur[p] for p in wB}, W_T, W_TN, tpsB, final=True)
                t2B, argB = gl_mid(wB, tpsB)
                npsA = {p: tgt.tile([128, 1024], F32, tag="tgt",
                                    name=f"nps{p}") for p in wA}
                gl_phase(argA, W_U, W_UN, npsA, final=False)
                gl_fin(wA, npsA, t2A)
                npsB = {p: tgt.tile([128, 1024], F32, tag="tgt",
                                    name=f"nps{p}") for p in wB}
                gl_phase(argB, W_U, W_UN, npsB, final=False)
                gl_fin(wB, npsB, t2B)

            # ---------- c_w / c1_w projections ----------
            for p in range(NPAIR):
                ps = tgt.tile([128, 1024], F32, tag="tgt")
                bdmm(ps, W_CW, prev[p])
                s00[p] = spool.tile([128, 2 * N], BF16, tag=f"s00_{p}", name=f"s00_{p}")
                drain(s00[p], ps, 1.0, BV_CB)
                ps = tgt.tile([128, 1024], F32, tag="tgt")
                bdmm(ps, W_C1W, cur[p])
                s11[p] = spool.tile([128, 2 * N], BF16, tag=f"s11_{p}", name=f"s11_{p}")
                drain(s11[p], ps, 1.0, BV_C1B)

            # ---------- NL chain (stage-interleaved over all pairs) ----------
            def nl_gates_stage(xs):
                """xs: {p: pair tile [128, 614]} -> {p: th [128, 4*307] bf16}."""
                wave = list(xs)
                ios = {p: scr.tile([128, 8 * N], BF16, tag="io_s", bufs=4,
                                   name=f"io{p}") for p in wave}
                gts = {p: scr.tile([128, 4 * N], BF16, tag="gt", bufs=4,
                                   name=f"gt{p}") for p in wave}
                for lo, hi, fn, outf_ in (
                    (W_GI_LO, W_GI_HI, AF.Sigmoid,
                     lambda p, g: ios[p][:, g * 4 * N:g * 4 * N + 2 * N]),
                    (W_GO_LO, W_GO_HI, AF.Sigmoid,
                     lambda p, g: ios[p][:, g * 4 * N + 2 * N:(g + 1) * 4 * N]),
                    (W_GG_LO, W_GG_HI, AF.Tanh,
                     lambda p, g: gts[p][:, g * 2 * N:(g + 1) * 2 * N]),
                ):
                    for p in wave:
                        for g in range(2):
                            xs_g = xs[p][:, g * N:(g + 1) * N]
                            ps = pp.tile([128, 1024], F32, tag="pp",
                                         name=f"gp{p}{g}")
                            nc.tensor.matmul(ps[:, 0:N], w(lo), xs_g,
                                             start=True, stop=True)
                            nc.tensor.matmul(ps[:, 512:512 + N], w(hi), xs_g,
                                             start=True, stop=True)
                            nc.scalar.activation(_rearr2(outf_(p, g)),
                                                 _rearr2(ps[:], N), fn)
                nw = len(wave)
                ch = scr.tile([128, nw * 4 * N], BF16, tag="c", bufs=2,
                              name="ch")
                for i, p in enumerate(wave):
                    i_ap = _rearr2(ios[p][:])[:, :, 0:2 * N]
                    nc.vector.tensor_mul(
                        _rearr2(ch[:, i * 4 * N:(i + 1) * 4 * N]), i_ap,
                        _rearr2(gts[p][:]))
                cth = scr.tile([128, nw * 4 * N], BF16, tag="ct", bufs=2,
                               name="cth")
                nc.scalar.activation(cth[:], ch[:], AF.Tanh)
                hhh = scr.tile([128, nw * 4 * N], BF16, tag="hh", bufs=2,
                               name="hhh")
                for i, p in enumerate(wave):
                    o_ap = _rearr2(ios[p][:])[:, :, 2 * N:4 * N]
                    nc.vector.tensor_mul(
                        _rearr2(hhh[:, i * 4 * N:(i + 1) * 4 * N]), o_ap,
                        _rearr2(cth[:, i * 4 * N:(i + 1) * 4 * N]))
                thh = scr.tile([128, nw * 4 * N], BF16, tag="th", bufs=2,
                               name="thh")
                nc.scalar.activation(thh[:], hhh[:], AF.Tanh)
                return {p: thh[:, i * 4 * N:(i + 1) * 4 * N]
                        for i, p in enumerate(wave)}

            def nl_w2mm(ps, th, lo, hi, stop=True):
                for g in range(2):
                    nc.tensor.matmul(ps[:, g * 512:g * 512 + N], w(lo),
                                     th[:, g * 2 * N:g * 2 * N + N],
                                     start=True, stop=False)
                    nc.tensor.matmul(ps[:, g * 512:g * 512 + N], w(hi),
                                     th[:, g * 2 * N + N:(g + 1) * 2 * N],
                                     start=False, stop=stop)

            for it in range(nl_iters):
                if it == 0:
                    xs = {p: s11[p] for p in range(NPAIR)}
                    basefs = {p: curf[p] for p in range(NPAIR)}
                    pargs = {p: s00[p] for p in range(NPAIR)}
                else:
                    xs = {p: gst[p] for p in range(NPAIR)}
                    basefs = {p: gstf[p] for p in range(NPAIR)}
                    pargs = {p: pprev[p] for p in range(NPAIR)}

                def nl_mid(half, ths):
                    """w2 matmuls + t2x drain + arg for a half (no ACT)."""
                    t2l, argl = {}, {}
                    for p in half:
                        tn = tgt.tile([128, 1024], F32, tag="tgt",
                                      name=f"tn{p}")
                        nl_w2mm(tn, ths[p], W_W2LO, W_W2HI)
                        t2l[p] = scr.tile([128, 2 * N], BF16, tag="t2x",
                                          bufs=5, name=f"t2xn{p}")
                        drain(t2l[p], tn, 2.0, BV_ZERO)       # 2*NL(x)
                    for p in half:
                        argl[p] = scr.tile([128, 2 * N], BF16, tag="arg",
                                           bufs=5, name=f"argn{p}")
                        nc.gpsimd.tensor_add(argl[p][:], pargs[p][:],
                                             t2l[p][:])
                    return t2l, argl

                def nl_fin(half, th2s, t2l):
                    """second-call w2 + fused combine drains + DMA (no ACT)."""
                    for p in half:
                        un = tgt.tile([128, 1024], F32, tag="tgt",
                                      name=f"un{p}")
                        nl_w2mm(un, th2s[p], W_W2LO5, W_W2HI5, stop=False)
                        bdmm(un, W_IQ, t2l[p], start=False, stop=True)
                        newf = states.tile([128, 2 * N], F32, tag=f"gf{p}",
                                           bufs=2, name=f"gf{p}")
                        nc.vector.tensor_add(_rearr2(newf[:]),
                                             _rearr2(un[:], N),
                                             _rearr2(basefs[p][:]))
                        new = states.tile([128, 2 * N], BF16, tag=f"g{p}",
                                          name=f"g{p}")
                        nc.vector.tensor_scalar(new[:], newf[:], 1.0, 0.0,
                                                ALU.mult, ALU.add)
                        nc.sync.dma_start(
                            d_out[it][:, p * 2 * N:(p + 1) * 2 * N], newf[:])
                        pprev[p] = xs[p]
                        gst[p], gstf[p] = new, newf

                h01, h23 = (0, 1), (2, 3)
                ths_01 = nl_gates_stage({p: xs[p] for p in h01})
                ths_23 = nl_gates_stage({p: xs[p] for p in h23})
                t2_01, arg_01 = nl_mid(h01, ths_01)
                th2_01 = nl_gates_stage(arg_01)
                t2_23, arg_23 = nl_mid(h23, ths_23)
                th2_23 = nl_gates_stage(arg_23)
                nl_fin(h01, th2_01, t2_01)
                nl_fin(h23, th2_23, t2_23)

        if bench_out:
            nc.sync.dma_start(d_outx[:], acc[:])

    nc.compile()
    return nc


def kernel(**inputs) -> np.ndarray:
    global _COMPILED
    consts = _host_consts(inputs)
    shards = _flow_shards(np.asarray(inputs["flow_x"], np.float32))
    if _COMPILED is None:
        _COMPILED = _build()
    nc = _COMPILED
    in_maps = []
    for c in range(NCORES):
        m = dict(consts)
        m["flowA"] = shards[c]
        in_maps.append(m)
    res = run_bass_kernel_spmd(nc, in_maps, core_ids=list(range(NCORES)))
    outs = []
    for c in range(NCORES):
        o = res.results[c]["out"]                     # [10, 128, 8*307]
        o = o.reshape(NL_ITERS, 8, 16, 8, N)          # [it, b_l, d, group, n]
        outs.append(np.ascontiguousarray(
            o.transpose(3, 1, 4, 0, 2).reshape(BL, N, NL_ITERS, H)))
    return np.concatenate(outs, axis=0).astype(np.float32)

